# revision 19
# baseline (speedup 1.0000x reference)
"""Multi-head Elman RNN (softsign) Trainium2 Bass kernel.

Problem: x[T,B,D] -> wx = einsum('hij,tbhj->tbhi', Wx, x.reshape(T,B,H,HD)) + bias
         scan over t: h = softsign(R @ h + wx_t);  outputs ys (=h_t stack) + h_final.
Shapes (hardcoded): T=4096, B=4, D=2048, H=64, HD=32, fp32.

Sharding: tensor-parallel over heads. 8 cores x 8 heads each; every core sees all
T and all B for its 8 heads (16 MB in / 16 MB out per core).

Per-core program (identical SPMD program, per-core data):
  Phase 1: DMA x slice in natural layout, transpose on PE to [d, (t,b)],
           blockdiag-128 matmuls compute wx for head-groups {0..3} (cols 0:4)
           and {4..7} (cols 4:8), ACT copies psum->SBUF "WX" buffer with bias,
           interleaved to layout [128 part = (head%4)*32+hd, T, 8].
  Phase 2: sequential scan. Per step: 8 self-loading 32x32 tile matmuls
           (R @ h_{t-1}) into one [128,8] psum tile, then softsign via
           3 DVE ops using  softsign(s) = rp - sign(rp),  rp ~= 1/(-s - sign(s)):
             op1 (custom):  dneg = -(psum + wx_t) - sign(psum + wx_t)
             op2 (stock) :  rp0  = reciprocal_approx_fast(dneg)
             op3 (custom):  rp1 = rp0*(2 - dneg*rp0);  h = rp1 - sign(rp1)
           h_t overwrites wx_t in WX (in-place state/output buffer).
  Phase 3: PE transposes WX back to natural rows, DMA out y; h_final from
           WX[:, T-1, :].
"""

import numpy as np

T, B, D = 4096, 4, 2048
H, HD = 64, 32
NCORES = 8
HPC = H // NCORES        # 8 heads per core
DPC = HPC * HD           # 256 dims per core
F32 = None               # set lazily (mybir.dt.float32)

_COMPILED = {}           # T -> (nc, names)


# --------------------------------------------------------------------------- #
# custom DVE ops
# --------------------------------------------------------------------------- #

def _register_dve_ops():
    import concourse.dve_ops as dve_ops
    from concourse.dve_spec import Spec, Src0, Src1, Zero, One, select, lower
    from concourse.dve_spec import _has_src1 as has_src1
    from concourse.dve_uop import DveOpSpec

    def reg(name, spec):
        if name in dve_ops._SUB_OPCODE_FOR_NAME:
            return next(op for op in dve_ops.OPS if op.name == name)
        shas = {}
        for ver in ("v3", "v4"):
            d = DveOpSpec(name=name, opcode=0, uops=lower(spec, ver=ver),
                          rd1_en=has_src1(spec))
            shas[ver] = d.sha(ver)
        op = dve_ops.DveOp(name, spec, subdim=False, uops_sha=shas)
        dve_ops.OPS.append(op)
        dve_ops.CUSTOM_DVE_SPECS[name] = spec
        dve_ops._SUB_OPCODE_FOR_NAME[name] = (
            dve_ops._CUSTOM_DVE_ROW_BASE + len(dve_ops.OPS) - 1)
        return op

    # op1: s = in0 + in1 ; out = -s - sign(s)   (sign(0) := +1)
    s = Src0 + Src1
    sg = select(s >= Zero, One, Zero - One)
    spec_dneg = Spec(
        body=(Zero - s) - sg,
        reference=lambda in0, in1, s0, s1, imm2:
            (lambda ss: (-ss - np.where(ss >= 0, 1.0, -1.0))
             .astype(np.float32))(in0 + in1),
    )

    # op3: rp1 = in1*(2 - in0*in1) ; out = rp1 - sign(rp1)
    t_ = Src0 * Src1
    rp1 = Src1 * ((One + One) - t_)
    sg3 = select(rp1 >= Zero, One, Zero - One)
    spec_fin = Spec(
        body=rp1 - sg3,
        reference=lambda in0, in1, s0, s1, imm2:
            (lambda r: (r - np.where(r >= 0, 1.0, -1.0))
             .astype(np.float32))(in1 * (2.0 - in0 * in1)),
    )

    return reg("ELMAN_DNEG", spec_dneg), reg("ELMAN_FIN", spec_fin)


# --------------------------------------------------------------------------- #
# kernel body (per-core program; identical across cores)
# --------------------------------------------------------------------------- #

def _build(T_, use_sim_t=None):
    import concourse.bass as bass
    import concourse.mybir as mybir
    import concourse.tile as tile
    from contextlib import ExitStack

    f32 = mybir.dt.float32
    Alu = mybir.AluOpType
    TB = T_ * B

    nc = bass.Bass("TRN2", target_bir_lowering=False, debug=False)

    # consts layout (free dim): WR[0:64] WXW[64:320] BIA[320:322] H0[322:330]
    #                           IDN[330:458]
    xs = nc.dram_tensor("xs", [TB, DPC], f32, kind="ExternalInput").ap()
    consts = nc.dram_tensor("consts", [128, 458], f32, kind="ExternalInput").ap()
    y = nc.dram_tensor("y", [TB, DPC], f32, kind="ExternalOutput").ap()
    hT = nc.dram_tensor("hT", [128, 8], f32, kind="ExternalOutput").ap()

    with tile.TileContext(nc) as tc, ExitStack() as ctx:
        sb = ctx.enter_context(tc.tile_pool(name="persist", bufs=1))
        psp = ctx.enter_context(tc.tile_pool(name="psp", bufs=1, space="PSUM"))

        CH = min(512, TB)              # rows per chunk (= CH//B t values)
        A = CH // 128
        nch = TB // CH
        NT = CH // B                   # t values per chunk

        WX = sb.tile([128, T_ * 8], f32, tag="WX")
        CONST = sb.tile([128, 458], f32, tag="CONST")
        xin = [sb.tile([128, A, DPC], f32, tag=f"xin{i}", name=f"xin{i}") for i in range(2)]
        xt = [sb.tile([128, 2, CH], f32, tag=f"xt{i}", name=f"xt{i}") for i in range(2)]
        stg = [sb.tile([128, 2, CH], f32, tag=f"stg{i}", name=f"stg{i}") for i in range(2)]
        yst = [sb.tile([128, A, 2, 128], f32, tag=f"yst{i}", name=f"yst{i}") for i in range(2)]
        dn = sb.tile([128, 8], f32, tag="dn")
        rp = sb.tile([128, 8], f32, tag="rp")
        sv = sb.tile([128, 8], f32, tag="sv")
        u1 = sb.tile([128, 8], f32, tag="u1")
        scr = sb.tile([128, 8], f32, tag="scr")

        ps_s = psp.tile([128, 8], f32, tag="ps_s")
        ps_tr = [psp.tile([128, 128], f32, tag=f"ps_tr{i}", name=f"ps_tr{i}") for i in range(2)]
        ps_dum = psp.tile([128, 128], f32, tag="ps_dum")
        ps_wx = [psp.tile([128, CH], f32, tag=f"ps_wx{i}", name=f"ps_wx{i}") for i in range(2)]

        WR = CONST[:, 0:64]
        WXW = CONST[:, 64:320]
        BIA = CONST[:, 320:322]
        H0 = CONST[:, 322:330]
        IDN = CONST[:, 330:458]

        nc.sync.dma_start(CONST[:], consts[:])
        Ident = mybir.ActivationFunctionType.Identity

        # one-off absorbers: make PE and ACT observe the consts DMA tick so
        # no later matmul carries that wait (1-wait-per-instruction limit).
        nc.tensor.transpose(ps_dum[:], IDN, IDN)
        nc.scalar.activation(scr[:], H0, Ident)

        # ------------------------- Phase 1: wx ---------------------------- #
        WXv = WX[:].rearrange("p (t e) -> p t e", e=8)
        for c in range(nch):
            xb = xin[c % 2]
            nc.sync.dma_start(
                xb[:],
                xs[c * CH:(c + 1) * CH, :].rearrange("(a p) f -> p a f", p=128))
            # dummy transpose: absorbs this chunk's DMA wait on the PE
            nc.tensor.transpose(ps_dum[0:32, :], xb[:, 0, 0:32], IDN)
            xtb = xt[c % 2]
            for a in range(A):
                for hf in range(2):
                    pst = ps_tr[hf]
                    nc.tensor.transpose(
                        pst[:], xb[:, a, hf * 128:(hf + 1) * 128], IDN)
                    nc.scalar.activation(
                        xtb[:, hf, a * 128:(a + 1) * 128], pst[:], Ident)
            t0 = c * NT
            for hf in range(2):
                pwx = ps_wx[hf]
                nc.tensor.matmul(
                    pwx[:], lhsT=WXW[:, hf * 128:(hf + 1) * 128],
                    rhs=xtb[:, hf, :], start=True, stop=True)
                nc.scalar.activation(
                    WXv[:, t0:t0 + NT, hf * 4:(hf + 1) * 4],
                    pwx[:].rearrange("p (t b) -> p t b", b=4),
                    Ident, bias=BIA[:, hf:hf + 1])

        # one-off: make DVE observe the ACT tick of the last phase-1 write
        # so scan DVE ops don't each carry an ACT wait.
        nc.vector.tensor_copy(scr[:], WX[:, (T_ - 1) * 8:T_ * 8])

        # ------------------------- Phase 2: scan -------------------------- #
        for t in range(T_):
            prev = H0 if t == 0 else WX[:, (t - 1) * 8:t * 8]
            for l in range(HPC):
                r, g = (l % 4) * 32, (l // 4)
                nc.tensor.matmul(
                    ps_s[r:r + 32, g * 4:(g + 1) * 4],
                    lhsT=WR[r:r + 32, g * 32:(g + 1) * 32],
                    rhs=prev[r:r + 32, g * 4:(g + 1) * 4],
                    start=True, stop=True, tile_position=(r, r))
            wx_t = WX[:, t * 8:(t + 1) * 8]
            # softsign(s) = s / (1 + |s|), 1+|s| = max(1+s, 1-s), s = psum+wx
            nc.vector.scalar_tensor_tensor(
                sv[:], ps_s[:], 1.0, wx_t, Alu.mult, Alu.add)      # s
            nc.vector.tensor_scalar(
                u1[:], sv[:], -1.0, 1.0, Alu.mult, Alu.add)        # 1 - s
            nc.vector.scalar_tensor_tensor(
                dn[:], sv[:], 1.0, u1[:], Alu.add, Alu.max)        # 1 + |s|
            nc.vector.reciprocal(rp[:], dn[:])
            nc.vector.tensor_mul(wx_t, sv[:], rp[:])               # h

        # ------------------------- Phase 3: output ------------------------ #
        nc.sync.dma_start(hT[:], WX[:, (T_ - 1) * 8:T_ * 8])
        for c in range(nch):
            tc0 = c * NT
            sg = stg[c % 2]
            for hf in range(2):
                nc.scalar.activation(
                    sg[:, hf, :],
                    WXv[:, tc0:tc0 + NT, hf * 4:(hf + 1) * 4], Ident)
            yb = yst[c % 2]
            if c >= 2:
                # absorb the y-DMA (HWDGE lane) wait on ACT so the yst
                # copies below carry only their PE wait
                nc.scalar.activation(scr[:], yb[:, 0, 0, 0:8], Ident)
            for a in range(A):
                for hf in range(2):
                    pst = ps_tr[hf]
                    nc.tensor.transpose(
                        pst[:], sg[:, hf, a * 128:(a + 1) * 128], IDN)
                    nc.scalar.activation(yb[:, a, hf, :], pst[:], Ident)
            nc.sync.dma_start(
                y[c * CH:(c + 1) * CH, :].rearrange(
                    "(a p) (h f) -> p a h f", p=128, h=2),
                yb[:])

    # This walrus build rejects the postamble EVENT_SEMAPHORE_RANGE_CLEAR
    # InstISA ("ISA wrong length" — bass_rust/walrus struct skew). It only
    # clears semaphores after the final drain+barrier; each NEFF execution
    # here is one-shot, so drop it.
    for f in nc.m.functions:
        for blk in f.blocks:
            blk.instructions = [
                i for i in blk.instructions
                if not (type(i).__name__ == "InstISA"
                        and getattr(i, "op_name", "") ==
                        "EVENT_SEMAPHORE_RANGE_CLEAR")
            ]
    _split_waits(nc, mybir)
    return nc


def _split_waits(nc, mybir):
    """This walrus accepts at most ONE sync-wait command per instruction.
    Tile emits semantically-minimal waits but can put several on one
    instruction. Split: hoist all but the last wait onto standalone
    InstEventSemaphore (pure-wait) instructions on the same engine queue,
    which execute in order before the real instruction."""
    n = 0
    for f in nc.m.functions:
        for blk in f.blocks:
            out = []
            for i in blk.instructions:
                si = getattr(i, "sync_info", None)
                ws = list(si.on_wait) if si and si.on_wait else []
                if len(ws) > 1:
                    for w in ws[:-1]:
                        n += 1
                        out.append(mybir.InstEventSemaphore(
                            name=f"WSPLIT-{n}",
                            engine=i.engine,
                            ins=[], outs=[],
                            sync_info=mybir.SyncInfo(
                                on_wait=[w], on_update=[]),
                        ))
                    i.sync_info = mybir.SyncInfo(
                        on_wait=[ws[-1]], on_update=list(si.on_update or []))
                out.append(i)
            blk.instructions = out


def _get_compiled(T_):
    if T_ not in _COMPILED:
        _COMPILED[T_] = _build(T_)
    return _COMPILED[T_]


# --------------------------------------------------------------------------- #
# host-side layout prep + dispatch
# --------------------------------------------------------------------------- #

def _scan_layout(a_bhli):
    """[B, HPC, HD] (one core's slice, head-local) -> [128, 8] scan layout."""
    out = np.zeros((128, 8), np.float32)
    for l in range(HPC):
        r, g = l % 4, l // 4
        # out[32r + i, 4g + b] = a[b, l, i]
        out[32 * r:32 * r + HD, 4 * g:4 * g + B] = a_bhli[:, l, :].T
    return out


def _make_in_maps(x, h0, R, Wx, bias, T_):
    x = np.ascontiguousarray(np.asarray(x, np.float32))
    h0 = np.asarray(h0, np.float32)
    R = np.asarray(R, np.float32)
    Wx = np.asarray(Wx, np.float32)
    bias = np.asarray(bias, np.float32)
    xr = x.reshape(T_ * B, H, HD)

    in_maps = []
    for k in range(NCORES):
        lo = k * HPC
        xs = np.ascontiguousarray(
            xr[:, lo:lo + HPC, :].reshape(T_ * B, DPC))
        consts = np.zeros((128, 458), np.float32)
        for l in range(HPC):
            r, g = l % 4, l // 4
            # lhsT[j, i] = R[l][i, j]
            consts[32 * r:32 * r + 32, 32 * g:32 * g + 32] = R[lo + l].T
        for l in range(4):
            consts[32 * l:32 * l + 32, 64 + 32 * l:64 + 32 * l + 32] = \
                Wx[lo + l].T
            consts[32 * l:32 * l + 32, 192 + 32 * l:192 + 32 * l + 32] = \
                Wx[lo + 4 + l].T
            consts[32 * l:32 * l + 32, 320] = bias[lo + l]
            consts[32 * l:32 * l + 32, 321] = bias[lo + 4 + l]
        consts[:, 322:330] = _scan_layout(h0[:, lo:lo + HPC, :])
        consts[:, 330:458] = np.eye(128, dtype=np.float32)
        in_maps.append({"xs": xs, "consts": consts})
    return in_maps


def _assemble(results, T_):
    output = np.empty((T_, B, H, HD), np.float32)
    h_final = np.empty((B, H, HD), np.float32)
    for k in range(NCORES):
        lo = k * HPC
        yk = results[k]["y"].reshape(T_, B, HPC, HD)
        output[:, :, lo:lo + HPC, :] = yk
        hTk = results[k]["hT"]
        for l in range(HPC):
            r, g = l % 4, l // 4
            h_final[:, lo + l, :] = hTk[32 * r:32 * r + HD, 4 * g:4 * g + B].T
    return output.reshape(T_, B, D), h_final


def _run(x, h0, R, Wx, bias, T_=T, trace=False):
    from concourse.bass_utils import run_bass_kernel_spmd
    nc = _get_compiled(T_)
    in_maps = _make_in_maps(x, h0, R, Wx, bias, T_)
    res = run_bass_kernel_spmd(nc, in_maps, list(range(NCORES)), trace=trace)
    out, h_final = _assemble(res.results, T_)
    return (out, h_final), res


def kernel(x, h0, R, Wx, bias):
    (out, h_final), _ = _run(x, h0, R, Wx, bias, T)
    return out, h_final


def _install_axon_ntff_hook():
    """Provide antenv.axon_hooks (absent in this image) so trace=True works."""
    import sys, types
    try:
        import antenv.axon_hooks  # noqa: F401
        return True
    except ImportError:
        pass
    try:
        from trn_agent_boot.trn_boot import _ntff_profile_via_ctypes
        hook = _ntff_profile_via_ctypes("/opt/axon/libaxon_pjrt.so")
        if hook is None:
            return False
        m = types.ModuleType("antenv.axon_hooks")
        m._hook = hook
        m.get_axon_ntff_profile_hook = lambda: m._hook
        m.set_axon_ntff_profile_hook = lambda h: setattr(m, "_hook", h)
        sys.modules["antenv.axon_hooks"] = m
        import antenv
        antenv.axon_hooks = m
        return True
    except Exception as e:  # pragma: no cover
        print("ntff hook install failed:", e)
        return False


def kernel_timed(x, h0, R, Wx, bias):
    import time
    traced = _install_axon_ntff_hook()
    try:
        (out, h_final), res = _run(x, h0, R, Wx, bias, T, trace=traced)
        if res.exec_time_ns is not None:
            return (out, h_final), res.exec_time_ns
    except Exception as e:
        print("traced run failed, falling back to wall clock:", e)
    # fallback: wall-clock the (NEFF-cached) second run
    t0 = time.time()
    (out, h_final), _ = _run(x, h0, R, Wx, bias, T, trace=False)
    return (out, h_final), int((time.time() - t0) * 1e9)


# revision 30
# speedup vs baseline: 1.1816x; 1.1816x over previous
"""Multi-head Elman RNN (softsign) Trainium2 Bass kernel.

Problem: x[T,B,D] -> wx = einsum('hij,tbhj->tbhi', Wx, x.reshape(T,B,H,HD)) + bias
         scan over t: h = softsign(R @ h + wx_t);  outputs ys (=h_t stack) + h_final.
Shapes (hardcoded): T=4096, B=4, D=2048, H=64, HD=32, fp32.

Sharding: tensor-parallel over heads. 8 cores x 8 heads each; every core sees all
T and all B for its 8 heads (16 MB in / 16 MB out per core).

Per-core program (identical SPMD program, per-core data):
  Phase 1: DMA x slice in natural layout, transpose on PE to [d, (t,b)],
           blockdiag-128 matmuls compute wx for head-groups {0..3} (cols 0:4)
           and {4..7} (cols 4:8), ACT copies psum->SBUF "WX" buffer with bias,
           interleaved to layout [128 part = (head%4)*32+hd, T, 8].
  Phase 2: sequential scan. Per step: 8 self-loading 32x32 tile matmuls
           (R @ h_{t-1}) into one [128,8] psum tile, then softsign via
           3 DVE ops using  softsign(s) = rp - sign(rp),  rp ~= 1/(-s - sign(s)):
             op1 (custom):  dneg = -(psum + wx_t) - sign(psum + wx_t)
             op2 (stock) :  rp0  = reciprocal_approx_fast(dneg)
             op3 (custom):  rp1 = rp0*(2 - dneg*rp0);  h = rp1 - sign(rp1)
           h_t overwrites wx_t in WX (in-place state/output buffer).
  Phase 3: PE transposes WX back to natural rows, DMA out y; h_final from
           WX[:, T-1, :].
"""

import numpy as np

T, B, D = 4096, 4, 2048
H, HD = 64, 32
NCORES = 8
HPC = H // NCORES        # 8 heads per core
DPC = HPC * HD           # 256 dims per core
F32 = None               # set lazily (mybir.dt.float32)

_COMPILED = {}           # T -> (nc, names)


# --------------------------------------------------------------------------- #
# custom DVE ops
# --------------------------------------------------------------------------- #

def _register_dve_ops():
    import concourse.dve_ops as dve_ops
    from concourse.dve_spec import Spec, Src0, Src1, Zero, One, select, lower
    from concourse.dve_spec import _has_src1 as has_src1
    from concourse.dve_uop import DveOpSpec

    def reg(name, spec):
        if name in dve_ops._SUB_OPCODE_FOR_NAME:
            return next(op for op in dve_ops.OPS if op.name == name)
        shas = {}
        for ver in ("v3", "v4"):
            d = DveOpSpec(name=name, opcode=0, uops=lower(spec, ver=ver),
                          rd1_en=has_src1(spec))
            shas[ver] = d.sha(ver)
        op = dve_ops.DveOp(name, spec, subdim=False, uops_sha=shas)
        dve_ops.OPS.append(op)
        dve_ops.CUSTOM_DVE_SPECS[name] = spec
        dve_ops._SUB_OPCODE_FOR_NAME[name] = (
            dve_ops._CUSTOM_DVE_ROW_BASE + len(dve_ops.OPS) - 1)
        return op

    # op1: s = in0 + in1 ; out = -s - sign(s)   (sign(0) := +1)
    s = Src0 + Src1
    sg = select(s >= Zero, One, Zero - One)
    spec_dneg = Spec(
        body=(Zero - s) - sg,
        reference=lambda in0, in1, s0, s1, imm2:
            (lambda ss: (-ss - np.where(ss >= 0, 1.0, -1.0))
             .astype(np.float32))(in0 + in1),
    )

    # op3: rp1 = in1*(2 - in0*in1) ; out = rp1 - sign(rp1)
    t_ = Src0 * Src1
    rp1 = Src1 * ((One + One) - t_)
    sg3 = select(rp1 >= Zero, One, Zero - One)
    spec_fin = Spec(
        body=rp1 - sg3,
        reference=lambda in0, in1, s0, s1, imm2:
            (lambda r: (r - np.where(r >= 0, 1.0, -1.0))
             .astype(np.float32))(in1 * (2.0 - in0 * in1)),
    )

    return reg("ELMAN_DNEG", spec_dneg), reg("ELMAN_FIN", spec_fin)


# --------------------------------------------------------------------------- #
# kernel body (per-core program; identical across cores)
# --------------------------------------------------------------------------- #

def _build(T_, split_waits=True):
    import concourse.bass as bass
    import concourse.mybir as mybir
    import concourse.tile as tile
    from contextlib import ExitStack

    f32 = mybir.dt.float32
    Alu = mybir.AluOpType
    TB = T_ * B

    nc = bass.Bass("TRN2", target_bir_lowering=False, debug=False)

    # consts layout (free dim): WR[0:64] WXW[64:320] BIA[320:322] H0[322:330]
    #                           IDN[330:458] ONES[458:466]
    xs = nc.dram_tensor("xs", [TB, DPC], f32, kind="ExternalInput").ap()
    consts = nc.dram_tensor("consts", [128, 466], f32, kind="ExternalInput").ap()
    y = nc.dram_tensor("y", [TB, DPC], f32, kind="ExternalOutput").ap()
    hT = nc.dram_tensor("hT", [128, 8], f32, kind="ExternalOutput").ap()

    with tile.TileContext(nc) as tc, ExitStack() as ctx:
        sb = ctx.enter_context(tc.tile_pool(name="persist", bufs=1))
        psp = ctx.enter_context(tc.tile_pool(name="psp", bufs=1, space="PSUM"))

        CH = min(512, TB)              # rows per chunk (= CH//B t values)
        A = CH // 128
        nch = TB // CH
        NT = CH // B                   # t values per chunk

        WX = sb.tile([128, T_ * 8], f32, tag="WX")
        CONST = sb.tile([128, 466], f32, tag="CONST")
        xin = [sb.tile([128, A, DPC], f32, tag=f"xin{i}", name=f"xin{i}") for i in range(2)]
        xt = [sb.tile([128, 2, CH], f32, tag=f"xt{i}", name=f"xt{i}") for i in range(2)]
        stg = [sb.tile([128, 2, CH], f32, tag=f"stg{i}", name=f"stg{i}") for i in range(2)]
        yst = [sb.tile([128, A, 2, 128], f32, tag=f"yst{i}", name=f"yst{i}") for i in range(2)]
        dn = sb.tile([128, 8], f32, tag="dn")
        u1s = sb.tile([128, 8], f32, tag="u1s")
        scr = sb.tile([128, 8], f32, tag="scr")

        ps_sp = [psp.tile([128, 512], f32, tag=f"ps_s{i}", name=f"ps_s{i}")
                 for i in range(2)]
        ps_tr = [psp.tile([128, 128], f32, tag=f"ps_tr{i}", name=f"ps_tr{i}") for i in range(2)]
        ps_dum = psp.tile([128, 128], f32, tag="ps_dum")
        ps_wx = [psp.tile([128, CH], f32, tag="ps_wx", name=f"ps_wx{i}", bufs=1)
                 for i in range(1)]

        WR = CONST[:, 0:64]
        WXW = CONST[:, 64:320]
        BIA = CONST[:, 320:322]
        H0 = CONST[:, 322:330]
        IDN = CONST[:, 330:458]
        ONES = CONST[:, 458:466]

        nc.sync.dma_start(CONST[:], consts[:])
        Ident = mybir.ActivationFunctionType.Identity

        # one-off absorbers: make PE, ACT and DVE observe the consts DMA
        # tick so no later instruction carries that wait (1-wait limit).
        nc.tensor.transpose(ps_dum[:], IDN, IDN)
        nc.scalar.activation(scr[:], H0, Ident)
        nc.vector.tensor_copy(scr[:], ONES)

        # ------------------------- Phase 1: wx ---------------------------- #
        WXv = WX[:].rearrange("p (t e) -> p t e", e=8)
        for c in range(nch):
            xb = xin[c % 2]
            nc.sync.dma_start(
                xb[:],
                xs[c * CH:(c + 1) * CH, :].rearrange("(a p) f -> p a f", p=128))
            # dummy transpose: absorbs this chunk's DMA wait on the PE
            nc.tensor.transpose(ps_dum[0:32, :], xb[:, 0, 0:32], IDN)
            xtb = xt[c % 2]
            for a in range(A):
                for hf in range(2):
                    pst = ps_tr[hf]
                    nc.tensor.transpose(
                        pst[:], xb[:, a, hf * 128:(hf + 1) * 128], IDN)
                    nc.scalar.activation(
                        xtb[:, hf, a * 128:(a + 1) * 128], pst[:], Ident)
            t0 = c * NT
            for hf in range(2):
                pwx = ps_wx[0]
                nc.tensor.matmul(
                    pwx[:], lhsT=WXW[:, hf * 128:(hf + 1) * 128],
                    rhs=xtb[:, hf, :], start=True, stop=True)
                nc.scalar.activation(
                    WXv[:, t0:t0 + NT, hf * 4:(hf + 1) * 4],
                    pwx[:].rearrange("p (t b) -> p t b", b=4),
                    Ident, bias=BIA[:, hf:hf + 1])
            # absorb this chunk's ACT tick on the PE so scan matmuls reading
            # WX carry no ACT wait (1-wait limit)
            nc.tensor.transpose(
                ps_dum[0:32, :], WX[:, (t0 + NT) * 8 - 32:(t0 + NT) * 8],
                IDN)

        # ------------------------- Phase 2: scan -------------------------- #
        for t in range(T_):
            if t % NT == 0:
                # absorb chunk (t//NT)'s phase-1 ACT tick on the DVE so the
                # in-place h writes into WX carry no ACT wait
                te = (t + NT - 1) * 8
                nc.vector.tensor_copy(scr[:], WX[:, te:te + 8])
            pb = ps_sp[t % 2]
            wx_t = WX[:, t * 8:(t + 1) * 8]
            # prefill psum with wx_t via identity matmuls (PE, off chain).
            # g=0 prefill is the bank's start=True (zeroes the 2KB row);
            # g=1 prefill lands on pending-zero bytes so start=False still
            # writes (not accumulates); R matmuls then accumulate.
            for l in range(HPC):
                r, g = (l % 4) * 32, (l // 4)
                nc.tensor.matmul(
                    pb[r:r + 32, g * 4:(g + 1) * 4],
                    lhsT=IDN[r:r + 32, r:r + 32],
                    rhs=wx_t[r:r + 32, g * 4:(g + 1) * 4],
                    start=(g == 0), stop=False, tile_position=(r, r),
                    skip_group_check=True)
            prev = H0 if t == 0 else WX[:, (t - 1) * 8:t * 8]
            for l in range(HPC):
                r, g = (l % 4) * 32, (l // 4)
                nc.tensor.matmul(
                    pb[r:r + 32, g * 4:(g + 1) * 4],
                    lhsT=WR[r:r + 32, g * 32:(g + 1) * 32],
                    rhs=prev[r:r + 32, g * 4:(g + 1) * 4],
                    start=False, stop=True, tile_position=(r, r),
                    skip_group_check=True)
            # softsign: h = s / (1+|s|);  s = psum;  1+|s| = max(1+s, 1-s)
            nc.vector.tensor_scalar(
                u1s[:], pb[:, 0:8], -1.0, 1.0, Alu.mult, Alu.add)   # 1 - s
            nc.vector.scalar_tensor_tensor(
                dn[:], pb[:, 0:8], 1.0, u1s[:], Alu.add, Alu.max)   # 1 + |s|
            nc.vector.reciprocal(scr[:], dn[:])
            nc.vector.tensor_mul(wx_t, pb[:, 0:8], scr[:])

        # ------------------------- Phase 3: output ------------------------ #
        nc.sync.dma_start(hT[:], WX[:, (T_ - 1) * 8:T_ * 8])
        for c in range(nch):
            tc0 = c * NT
            sg = stg[c % 2]
            for hf in range(2):
                nc.scalar.activation(
                    sg[:, hf, :],
                    WXv[:, tc0:tc0 + NT, hf * 4:(hf + 1) * 4], Ident)
            yb = yst[c % 2]
            if c >= 2:
                # absorb the y-DMA (HWDGE lane) wait on ACT so the yst
                # copies below carry only their PE wait
                nc.scalar.activation(scr[:], yb[:, 0, 0, 0:8], Ident)
            for a in range(A):
                for hf in range(2):
                    pst = ps_tr[hf]
                    nc.tensor.transpose(
                        pst[:], sg[:, hf, a * 128:(a + 1) * 128], IDN)
                    nc.scalar.activation(yb[:, a, hf, :], pst[:], Ident)
            nc.sync.dma_start(
                y[c * CH:(c + 1) * CH, :].rearrange(
                    "(a p) (h f) -> p a h f", p=128, h=2),
                yb[:])

    # This walrus build rejects the postamble EVENT_SEMAPHORE_RANGE_CLEAR
    # InstISA ("ISA wrong length" — bass_rust/walrus struct skew). It only
    # clears semaphores after the final drain+barrier; each NEFF execution
    # here is one-shot, so drop it.
    for f in nc.m.functions:
        for blk in f.blocks:
            blk.instructions = [
                i for i in blk.instructions
                if not (type(i).__name__ == "InstISA"
                        and getattr(i, "op_name", "") ==
                        "EVENT_SEMAPHORE_RANGE_CLEAR")
            ]
    if split_waits:
        _split_waits(nc, mybir)
    return nc


def _split_waits(nc, mybir):
    """This walrus accepts at most ONE sync-wait command per instruction.
    Tile emits semantically-minimal waits but can put several on one
    instruction. First drop same-engine waits (engine executes its queue in
    order, and Tile only emits them for queue-ahead throttling; the data
    hazard is already covered transitively). Then split: hoist all but the
    last wait onto standalone InstEventSemaphore (pure-wait) instructions on
    the same engine queue, which execute in order before the real one."""
    eng_prefix = {
        "PE": "PE_", "DVE": "DVE_", "Activation": "Activation_",
        "Pool": "Pool_", "SP": "SP_",
    }
    n = 0
    for f in nc.m.functions:
        for blk in f.blocks:
            out = []
            for i in blk.instructions:
                si = getattr(i, "sync_info", None)
                ws = list(si.on_wait) if si and si.on_wait else []
                pfx = eng_prefix.get(getattr(i.engine, "name", str(i.engine)))
                if len(ws) > 1 and pfx:
                    keep = [w for w in ws
                            if not (w.ant_name or "").startswith(pfx)]
                    if keep:
                        ws = keep
                        i.sync_info = mybir.SyncInfo(
                            on_wait=list(ws),
                            on_update=list(si.on_update or []))
                        si = i.sync_info
                if len(ws) > 1:
                    for w in ws[:-1]:
                        n += 1
                        out.append(mybir.InstEventSemaphore(
                            name=f"WSPLIT-{n}",
                            engine=i.engine,
                            ins=[], outs=[],
                            sync_info=mybir.SyncInfo(
                                on_wait=[w], on_update=[]),
                        ))
                    i.sync_info = mybir.SyncInfo(
                        on_wait=[ws[-1]], on_update=list(si.on_update or []))
                out.append(i)
            blk.instructions = out


def _get_compiled(T_):
    if T_ not in _COMPILED:
        _COMPILED[T_] = _build(T_)
    return _COMPILED[T_]


# --------------------------------------------------------------------------- #
# host-side layout prep + dispatch
# --------------------------------------------------------------------------- #

def _scan_layout(a_bhli):
    """[B, HPC, HD] (one core's slice, head-local) -> [128, 8] scan layout."""
    out = np.zeros((128, 8), np.float32)
    for l in range(HPC):
        r, g = l % 4, l // 4
        # out[32r + i, 4g + b] = a[b, l, i]
        out[32 * r:32 * r + HD, 4 * g:4 * g + B] = a_bhli[:, l, :].T
    return out


def _make_in_maps(x, h0, R, Wx, bias, T_):
    x = np.ascontiguousarray(np.asarray(x, np.float32))
    h0 = np.asarray(h0, np.float32)
    R = np.asarray(R, np.float32)
    Wx = np.asarray(Wx, np.float32)
    bias = np.asarray(bias, np.float32)
    xr = x.reshape(T_ * B, H, HD)

    in_maps = []
    for k in range(NCORES):
        lo = k * HPC
        xs = np.ascontiguousarray(
            xr[:, lo:lo + HPC, :].reshape(T_ * B, DPC))
        consts = np.zeros((128, 466), np.float32)
        for l in range(HPC):
            r, g = l % 4, l // 4
            # lhsT[j, i] = R[l][i, j]
            consts[32 * r:32 * r + 32, 32 * g:32 * g + 32] = R[lo + l].T
        for l in range(4):
            consts[32 * l:32 * l + 32, 64 + 32 * l:64 + 32 * l + 32] = \
                Wx[lo + l].T
            consts[32 * l:32 * l + 32, 192 + 32 * l:192 + 32 * l + 32] = \
                Wx[lo + 4 + l].T
            consts[32 * l:32 * l + 32, 320] = bias[lo + l]
            consts[32 * l:32 * l + 32, 321] = bias[lo + 4 + l]
        consts[:, 322:330] = _scan_layout(h0[:, lo:lo + HPC, :])
        consts[:, 330:458] = np.eye(128, dtype=np.float32)
        consts[:, 458:466] = 1.0
        in_maps.append({"xs": xs, "consts": consts})
    return in_maps


def _assemble(results, T_):
    output = np.empty((T_, B, H, HD), np.float32)
    h_final = np.empty((B, H, HD), np.float32)
    for k in range(NCORES):
        lo = k * HPC
        yk = results[k]["y"].reshape(T_, B, HPC, HD)
        output[:, :, lo:lo + HPC, :] = yk
        hTk = results[k]["hT"]
        for l in range(HPC):
            r, g = l % 4, l // 4
            h_final[:, lo + l, :] = hTk[32 * r:32 * r + HD, 4 * g:4 * g + B].T
    return output.reshape(T_, B, D), h_final


def _run(x, h0, R, Wx, bias, T_=T, trace=False):
    from concourse.bass_utils import run_bass_kernel_spmd
    nc = _get_compiled(T_)
    in_maps = _make_in_maps(x, h0, R, Wx, bias, T_)
    res = run_bass_kernel_spmd(nc, in_maps, list(range(NCORES)), trace=trace)
    out, h_final = _assemble(res.results, T_)
    return (out, h_final), res


def kernel(x, h0, R, Wx, bias):
    (out, h_final), _ = _run(x, h0, R, Wx, bias, T)
    return out, h_final


def _install_axon_ntff_hook():
    """Provide antenv.axon_hooks (absent in this image) so trace=True works."""
    import sys, types
    try:
        import antenv.axon_hooks  # noqa: F401
        return True
    except ImportError:
        pass
    try:
        from trn_agent_boot.trn_boot import _ntff_profile_via_ctypes
        hook = _ntff_profile_via_ctypes("/opt/axon/libaxon_pjrt.so")
        if hook is None:
            return False
        m = types.ModuleType("antenv.axon_hooks")
        m._hook = hook
        m.get_axon_ntff_profile_hook = lambda: m._hook
        m.set_axon_ntff_profile_hook = lambda h: setattr(m, "_hook", h)
        sys.modules["antenv.axon_hooks"] = m
        import antenv
        antenv.axon_hooks = m
        return True
    except Exception as e:  # pragma: no cover
        print("ntff hook install failed:", e)
        return False


def kernel_timed(x, h0, R, Wx, bias):
    import time
    traced = _install_axon_ntff_hook()
    try:
        (out, h_final), res = _run(x, h0, R, Wx, bias, T, trace=traced)
        if res.exec_time_ns is not None:
            return (out, h_final), res.exec_time_ns
    except Exception as e:
        print("traced run failed, falling back to wall clock:", e)
    # fallback: wall-clock the (NEFF-cached) second run
    t0 = time.time()
    (out, h_final), _ = _run(x, h0, R, Wx, bias, T, trace=False)
    return (out, h_final), int((time.time() - t0) * 1e9)
